# revision 40
# baseline (speedup 1.0000x reference)
"""Trainium2 Bass kernel for nn_Attention (dense transformer attention w/ QK-LayerNorm).

Sharding: sequence-parallel over 8 cores. Core c handles batch b = c//2,
token half h = c%2 (512 tokens). K/V are computed redundantly for the full
batch element on both cores of a pair; Q only for local tokens. No collectives.

Structure (v4):
 - bf16 on the SBUF side everywhere (weights, x, K/Q/V, probs, output);
   PSUM accumulation fp32. Halves DMA bytes, enables fast DVE modes + FWL.
 - Inputs arrive as 13 large DMAs (x, 2 halves per weight tensor); K/Q run
   kt-outer over 4-bank accumulator half-waves so matmuls start as soon as
   the first half-tensor lands (~7us) instead of after the full load.
 - All three LayerNorm stat chains share ONE Ln + ONE Exp ACT call -> no
   activation-table thrash against the attention-phase Exp stream.
 - Softmax exp: one ACT call per key-tile over a 2-bank [128,1024] PSUM pair
   (both heads of the pair at once).
 - Denominators: V carries a trailing ones-column per head, PV emits row 64 =
   sum(P); per-head-pair DVE reciprocal_approx_fast (at partition 0 only -
   the custom op miscompiles at partition base != 0) + GpSimd
   partition_broadcast of 1/d; osb scaled in-loop so the projection starts
   right after the last head pair.
 - PSUM: QKV scope = 4-bank acc rotation + {ssqK0,ssqK1,ssqQ,bcast};
   attention scope = 2x2-bank S pairs + O banks; proj scope = 4 banks.

Engine partition rule: writes and ACT reads start at 32-aligned partitions;
only DVE reads use unaligned bases.
"""

import numpy as np

B, N, C = 4, 1024, 1024
H, D = 16, 64
LN_EPS = 1e-5
N_CORES = 8
TL = 512          # local tokens per core
KT = 8            # channel tiles of 128
SCALE = D ** -0.5

_COMPILED = None


def _build():
    import concourse.bacc as bacc
    import concourse.tile as tile
    import concourse.mybir as mybir

    F32 = mybir.dt.float32
    F32R = mybir.dt.float32r
    BF16 = mybir.dt.bfloat16
    I16 = mybir.dt.int16
    AF = mybir.ActivationFunctionType
    OP = mybir.AluOpType
    # Schraudolph exp in bf16-bit space: int16(A*S + B) bitcast to bf16
    # approximates exp(SCALE*S); the constant-scale part of the error
    # cancels in the softmax normalization.
    EXPA = 128.0 * 1.4426950408889634 * SCALE
    EXPB = 128.0 * (127.0 - 0.0434)

    nc = bacc.Bacc("TRN2", target_bir_lowering=False, debug=False,
                   num_devices=N_CORES)

    xT_d = nc.dram_tensor("xT", [C, N], BF16, kind="ExternalInput").ap()
    qkvwT_d = nc.dram_tensor("qkvwT", [C, 3 * C], BF16, kind="ExternalInput").ap()
    projwT_d = nc.dram_tensor("projwT", [C, C], BF16, kind="ExternalInput").ap()
    constf_d = nc.dram_tensor("constf", [128, 58], F32, kind="ExternalInput").ap()
    constb_d = nc.dram_tensor("constb", [128, 20], BF16, kind="ExternalInput").ap()
    selc_d = nc.dram_tensor("selc", [2, 256], F32R, kind="ExternalInput").ap()
    out_d = nc.dram_tensor("out", [C, TL], BF16, kind="ExternalOutput").ap()

    with tile.TileContext(nc) as tc:
        with tc.tile_pool(name="persist", bufs=1) as pers, \
             tc.tile_pool(name="sq", bufs=2) as sqp, \
             tc.tile_pool(name="small", bufs=1) as smallp:

            khat = pers.tile([128, KT * N], BF16, tag="khat")       # 2MB
            vful = pers.tile([128, 8 * 1040], BF16, tag="vful")     # 2.03MB
            qhat = pers.tile([128, KT * TL], BF16, tag="qhat")      # 1MB
            osb = pers.tile([128, KT * TL], BF16, tag="osb")        # 1MB
            wpbig = pers.tile([128, KT * 1024], BF16, tag="wpbig")  # 2MB

            constf = smallp.tile([128, 58], F32, tag="constf")
            constb = smallp.tile([128, 20], BF16, tag="constb")
            selc = smallp.tile([2, 256], F32R, tag="selc")
            st = smallp.tile([2, 8 * 512], F32, tag="st")
            stmm = smallp.tile([2, 6 * 512], F32R, tag="stmm")
            rstdb = smallp.tile([128, 2 * N + 2 * TL], BF16, tag="rstdb")
            dd = smallp.tile([1, 1024], F32, tag="dd")
            ddr = smallp.tile([1, 1024], F32, tag="ddr")
            ddrb = smallp.tile([1, 1024], BF16, tag="ddrb")
            qzs = [smallp.tile([128, 1024], BF16, tag=f"qz{i}",
                               name=f"qz{i}") for i in range(2)]

            bsum = constf[0:2, 56:57]
            wsums = constb[:, 0:16]
            sel = constb[:, 16:20]    # col0 = sum-to-row-0 ; col3 = sum-to-row-1

            def sl(i):
                return st[:, i * 512:(i + 1) * 512]

            def smm(i):
                return stmm[:, i * 512:(i + 1) * 512]

            def prm(grp, kt):  # qn_w qn_b kn_w kn_b qb kb projb
                return constf[:, grp * 8 + kt: grp * 8 + kt + 1]

            nc.sync.dma_start(constf[:], constf_d[:])
            nc.sync.dma_start(constb[:], constb_d[:])
            nc.sync.dma_start(selc[:], selc_d[:])
            for q in qzs:
                nc.vector.memset(q[:], 0.0)

            with tc.tile_pool(name="xp", bufs=1) as xpool, \
                 tc.tile_pool(name="ntmp", bufs=3) as ntp, \
                 tc.tile_pool(name="qkvps", bufs=1, space="PSUM") as qps, \
                 tc.tile_pool(name="statps", bufs=1, space="PSUM") as sps:
                xT = xpool.tile([128, KT * N], BF16, tag="xT")      # 2MB
                wkbig = xpool.tile([128, KT * 1024], BF16, tag="wkbig")
                wqbig = xpool.tile([128, KT * 1024], BF16, tag="wqbig")
                wvbig = xpool.tile([128, KT * 1024], BF16, tag="wvbig")

                # ---- input DMA: xT and the K weights arrive in kt-halves,
                # interleaved, so the kt-outer K waves start on the first
                # 1.5MB instead of the full load; later tensors in g-halves
                xdst = xT[:].rearrange("p (a t) -> p a t", a=KT)
                xsrc = xT_d.rearrange("(a p) t -> p a t", p=128)
                wkdst = wkbig[:].rearrange("p (a t) -> p a t", a=KT)
                wksrc = qkvwT_d[:, C:2 * C].rearrange("(a p) t -> p a t",
                                                      p=128)

                def wload(big, col0):
                    for g in range(2):
                        dst = big[:].rearrange("p (a t) -> p a t", a=KT)
                        nc.sync.dma_start(
                            dst[:, :, g * 512:(g + 1) * 512],
                            qkvwT_d[:, col0 + g * 512: col0 + (g + 1) * 512]
                            .rearrange("(a p) t -> p a t", p=128))

                for h in range(2):
                    ks = slice(h * 4, (h + 1) * 4)
                    nc.sync.dma_start(xdst[:, ks, :], xsrc[:, ks, :])
                    nc.sync.dma_start(
                        wkdst[:, ks, 0:512], wksrc[:, ks, 0:512])
                for h in range(2):
                    ks = slice(h * 4, (h + 1) * 4)
                    nc.sync.dma_start(
                        wkdst[:, ks, 512:1024], wksrc[:, ks, 512:1024])
                wload(wqbig, 0)
                wload(wvbig, 2 * C)
                for g in range(2):
                    dst = wpbig[:].rearrange("p (a t) -> p a t", a=KT)
                    nc.sync.dma_start(
                        dst[:, :, g * 512:(g + 1) * 512],
                        projwT_d[:, g * 512:(g + 1) * 512]
                        .rearrange("(a p) t -> p a t", p=128))

                # ---- K phase: kt-outer half-waves of 4 accumulators ----
                def kq_wave(big, grp, moving0, mwidth, dst, dstep, bias_g):
                    accs = [qps.tile([128, mwidth], F32, tag="acc", bufs=4,
                                     name=f"acc{grp}_{mi}") for mi in range(4)]
                    for kt in range(KT):
                        for mi in range(4):
                            nc.tensor.matmul(
                                accs[mi][:],
                                big[:, kt * 1024 + grp * 512 + mi * 128:
                                    kt * 1024 + grp * 512 + (mi + 1) * 128],
                                xT[:, kt * N + moving0: kt * N + moving0 + mwidth],
                                start=(kt == 0), stop=(kt == KT - 1),
                                skip_group_check=True)
                    for mi in range(4):
                        m = grp * 4 + mi
                        nc.scalar.activation(
                            dst(m), accs[mi][:], AF.Identity,
                            bias=prm(bias_g, m))

                for grp in range(2):
                    for nh in range(2):
                        kq_wave(wkbig, grp, nh * 512, 512,
                                lambda m, nh=nh: khat[:, m * N + nh * 512:
                                                      m * N + (nh + 1) * 512],
                                512, 5)

                # ---- token sums of (q,k): needs only xT ----
                sums_ps = [qps.tile([2, 512], F32, tag="acc", bufs=4,
                                    name=f"sums_ps{i}") for i in range(2)]
                for kt in range(KT):
                    for nh in range(2):
                        nc.tensor.matmul(
                            sums_ps[nh][:], wsums[:, kt * 2: kt * 2 + 2],
                            xT[:, kt * N + nh * 512: kt * N + (nh + 1) * 512],
                            start=(kt == 0), stop=(kt == KT - 1),
                            skip_group_check=True)
                nc.vector.tensor_copy(sl(0), sums_ps[0][:])
                nc.vector.tensor_copy(sl(1), sums_ps[1][:])

                for grp in range(2):
                    kq_wave(wqbig, grp, 0, TL,
                            lambda m: qhat[:, m * TL:(m + 1) * TL], 512, 4)

                # ---- ssq matmuls (squares on DVE, overlap Q/V matmuls) ----
                ssq_ps = [sps.tile([2, 512], F32, tag=f"sq{i}", bufs=1,
                                   name=f"ssq_ps{i}") for i in range(2)]
                for m in range(8):
                    s = khat[:, m * N:(m + 1) * N]
                    ksq = sqp.tile([128, N], BF16, tag="sq")
                    nc.vector.tensor_mul(ksq[:], s, s)
                    for nh in range(2):
                        nc.tensor.matmul(
                            ssq_ps[nh][:], sel[:, 2:4],
                            ksq[:, nh * 512:(nh + 1) * 512],
                            start=(m == 0), stop=(m == 7),
                            skip_group_check=True)
                ssqQ_ps = sps.tile([2, 512], F32, tag="sqQ", bufs=1,
                                   name="ssqQ_ps")
                for m in range(8):
                    s = qhat[:, m * TL:(m + 1) * TL]
                    qsq = sqp.tile([128, TL], BF16, tag="sqq")
                    nc.vector.tensor_mul(qsq[:], s, s)
                    nc.tensor.matmul(ssqQ_ps[:], sel[:, 0:2], qsq[:],
                                     start=(m == 0), stop=(m == 7),
                                     skip_group_check=True)

                # ---- stats: one batched Ln+Exp for all three chains ----
                # slots: mu_a=sl(2) [nh0], mu_b=sl(3) [nh1]; vars at sl(4..6)
                # (nh0-k, nh1-k, q); rstd_cat -> smm(0..2), murs -> smm(3..5)
                nc.vector.tensor_scalar(sl(2), sl(0), 1.0 / C, bsum,
                                        OP.mult, OP.add)
                nc.vector.tensor_scalar(sl(3), sl(1), 1.0 / C, bsum,
                                        OP.mult, OP.add)

                def var_into(slot, ssq_src, mu):
                    nc.vector.tensor_copy(sl(7), ssq_src)
                    nc.vector.tensor_scalar(slot, sl(7), 1.0 / C, LN_EPS,
                                            OP.mult, OP.add)
                    nc.vector.tensor_mul(sl(7), mu, mu)
                    nc.vector.tensor_sub(slot, slot, sl(7))

                var_into(sl(4), ssq_ps[0][:], sl(2))
                var_into(sl(5), ssq_ps[1][:], sl(3))
                var_into(sl(6), ssqQ_ps[:], sl(2))
                vcat = st[:, 4 * 512:7 * 512]
                nc.vector.tensor_scalar_max(vcat, vcat, 1e-20)
                nc.scalar.activation(vcat, vcat, AF.Ln)
                nc.scalar.activation(stmm[:, 0:3 * 512], vcat, AF.Exp,
                                     scale=-0.5)
                nc.vector.tensor_mul(smm(3), sl(2), smm(0).bitcast(F32))
                nc.vector.tensor_mul(smm(4), sl(3), smm(1).bitcast(F32))
                nc.vector.tensor_mul(smm(5), sl(2), smm(2).bitcast(F32))

                def bcast(slot, row, dst_col):
                    bc_ps = sps.tile([128, 512], F32, tag="bc", bufs=1,
                                     name="bc_ps")
                    nc.tensor.matmul(bc_ps[:],
                                     selc[0:2, row * 128:(row + 1) * 128],
                                     smm(slot), start=True, stop=True)
                    nc.vector.tensor_copy(rstdb[:, dst_col:dst_col + 512],
                                          bc_ps[:])

                bcast(0, 1, 0)        # rstd_k nh0
                bcast(3, 1, N)        # murs_k nh0
                bcast(1, 1, 512)      # rstd_k nh1
                bcast(4, 1, N + 512)  # murs_k nh1
                bcast(2, 0, 2 * N)    # rstd_q
                bcast(5, 0, 2 * N + TL)

                # ---- V phase (kt-inner, acc rotation) ----
                for nh in range(2):
                    for mt in range(8):
                        base = mt * 1040
                        acc = qps.tile([128, 512], F32, tag="acc", bufs=4,
                                       name="vacc")
                        for kt in range(KT):
                            nc.tensor.matmul(
                                acc[:],
                                xT[:, kt * N + mt * 128: kt * N + (mt + 1) * 128],
                                wvbig[:, kt * 1024 + nh * 512:
                                      kt * 1024 + (nh + 1) * 512],
                                start=(kt == 0), stop=(kt == KT - 1))
                        # head slot layout: [64 V chans | ones]
                        dst = vful[:, base + nh * 8 * 65: base + (nh + 1) * 8 * 65]
                        nc.scalar.activation(
                            dst.rearrange("p (h e) -> p h e", h=8)[:, :, 0:64],
                            acc[:].rearrange("p (h e) -> p h e", h=8),
                            AF.Copy)
                for mt in range(8):
                    oc = vful[:, mt * 1040: (mt + 1) * 1040]
                    oc = oc.rearrange("p (h e) -> p h e", h=16)[:, :, 64:65]
                    nc.vector.memset(oc, 1.0)

                # ---- normalize K and Q (DVE+ACT, overlaps V matmuls) ----
                for m in range(8):
                    s = khat[:, m * N:(m + 1) * N]
                    t = ntp.tile([128, N], BF16, tag="nt")
                    nc.vector.tensor_mul(t[:], s, rstdb[:, 0:N])
                    nc.vector.tensor_sub(t[:], t[:], rstdb[:, N:2 * N])
                    nc.scalar.activation(s, t[:], AF.Identity,
                                         scale=prm(2, m), bias=prm(3, m))
                for m in range(8):
                    s = qhat[:, m * TL:(m + 1) * TL]
                    t = ntp.tile([128, TL], BF16, tag="ntq")
                    nc.vector.tensor_mul(t[:], s, rstdb[:, 2 * N:2 * N + TL])
                    nc.vector.tensor_sub(
                        t[:], t[:], rstdb[:, 2 * N + TL:2 * N + 2 * TL])
                    nc.scalar.activation(s, t[:], AF.Identity,
                                         scale=prm(0, m), bias=prm(1, m))

            # ---------- attention (head pairs, software-pipelined) ----------
            with tc.tile_pool(name="pp", bufs=3) as ppool, \
                 tc.tile_pool(name="att", bufs=2) as attp, \
                 tc.tile_pool(name="ot", bufs=2) as otp, \
                 tc.tile_pool(name="attps", bufs=1, space="PSUM") as aps:

                def qz_copy(kth):
                    qz = qzs[kth % 2]
                    nc.vector.tensor_copy(
                        qz[0:64, 0:512], qhat[0:64, kth * TL:(kth + 1) * TL])
                    nc.vector.tensor_copy(
                        qz[64:128, 512:1024],
                        qhat[64:128, kth * TL:(kth + 1) * TL])

                qz_copy(0)
                qz_copy(1)
                o_pss = {}
                rbs = {}

                def scale_osb(kth):
                    # fused drain+scale of head pair kth: reads its O banks
                    # (freeing them for kth+2) and its 1/d broadcast, which
                    # by now has had a full iteration to land
                    oA, oB = o_pss.pop(kth)
                    rb = rbs.pop(kth)
                    sl_o = osb[:, kth * TL:(kth + 1) * TL]
                    nc.vector.scalar_tensor_tensor(
                        sl_o[0:64, :], oA[0:64, :], 1.0, rb[0:64, 0:512],
                        OP.mult, OP.mult)
                    nc.vector.scalar_tensor_tensor(
                        sl_o[64:128, :], oB[0:64, :], 1.0,
                        rb[64:128, 512:1024], OP.mult, OP.mult)

                for kth in range(8):
                    hA, hB = 2 * kth, 2 * kth + 1
                    qz = qzs[kth % 2]
                    o_psA = aps.tile([65, 512], F32, tag="oA", bufs=2,
                                     name=f"oA{kth}")
                    o_psB = aps.tile([65, 512], F32, tag="oB", bufs=2,
                                     name=f"oB{kth}")

                    s_tiles = {}

                    def emit_S(tt, qz=qz, kth=kth, s_tiles=s_tiles):
                        ksl = khat[:, kth * N + tt * 128: kth * N + (tt + 1) * 128]
                        sp = aps.tile([128, 1024], F32, tag="sp", bufs=2,
                                      name=f"sp{tt}")
                        nc.tensor.matmul(sp[:, 0:512], ksl, qz[:, 0:512],
                                         start=True, stop=True)
                        nc.tensor.matmul(sp[:, 512:1024], ksl, qz[:, 512:1024],
                                         start=True, stop=True)
                        s_tiles[tt] = sp

                    emit_S(0)
                    emit_S(1)
                    for tt in range(8):
                        sp = s_tiles.pop(tt)
                        if tt in (2, 5):
                            # offload 2 of 8 exp tiles to DVE (Schraudolph)
                            pi = ppool.tile([128, 1024], I16, tag="p",
                                            name="pei")
                            nc.vector.tensor_scalar(pi[:], sp[:], EXPA, EXPB,
                                                    OP.mult, OP.add)
                            pv = pi[:].bitcast(BF16)
                        else:
                            pe = ppool.tile([128, 1024], BF16, tag="p",
                                            name="pe")
                            nc.scalar.activation(pe[:], sp[:], AF.Exp,
                                                 scale=SCALE)
                            pv = pe[:]
                        if tt + 2 < 8:
                            emit_S(tt + 2)
                        nc.tensor.matmul(
                            o_psA[:],
                            vful[:, tt * 1040 + hA * 65: tt * 1040 + (hA + 1) * 65],
                            pv[:, 0:512], start=(tt == 0), stop=(tt == 7))
                        nc.tensor.matmul(
                            o_psB[:],
                            vful[:, tt * 1040 + hB * 65: tt * 1040 + (hB + 1) * 65],
                            pv[:, 512:1024], start=(tt == 0), stop=(tt == 7))

                    # prefetch next-next qz so S(kth+1) never waits on DVE
                    if kth + 2 < 8:
                        qz_copy(kth + 2)
                    # denominators -> 1/d (DVE) -> broadcast (GpSimd)
                    nc.vector.tensor_copy(dd[0:1, 0:512], o_psA[64:65, :])
                    nc.vector.tensor_copy(dd[0:1, 512:1024], o_psB[64:65, :])
                    nc.vector.reciprocal_approx_fast(ddr[0:1, :], dd[0:1, :])
                    nc.vector.tensor_copy(ddrb[:], ddr[:])
                    rb = attp.tile([128, 1024], BF16, tag="rb")
                    nc.gpsimd.partition_broadcast(rb[:], ddrb[0:1, :])
                    o_pss[kth] = (o_psA, o_psB)
                    rbs[kth] = rb
                    if kth > 0:
                        scale_osb(kth - 1)
                scale_osb(7)

                # ---------- output projection (same PSUM scope: the acc
                # tiles recycle the sp/oA/oB bank slots, so no scope barrier
                # separates the last head pair from the projection) ----------
                for wave in range(2):
                    accs = [aps.tile([128, 512], F32, tag=t, bufs=b,
                                     name=f"pacc{wave}_{mi}")
                            for mi, (t, b) in enumerate(
                                (("sp", 2), ("sp", 2), ("oA", 2), ("oB", 2)))]
                    for kth in range(8):
                        for mi in range(4):
                            m = wave * 4 + mi
                            nc.tensor.matmul(
                                accs[mi][:],
                                wpbig[:, kth * 1024 + (m // 4) * 512 + (m % 4) * 128:
                                      kth * 1024 + (m // 4) * 512 + (m % 4 + 1) * 128],
                                osb[:, kth * TL:(kth + 1) * TL],
                                start=(kth == 0), stop=(kth == 7),
                                skip_group_check=True)
                    ow = otp.tile([128, 4 * 512], BF16, tag="ot",
                                  name=f"ow{wave}")
                    for mi in range(4):
                        m = wave * 4 + mi
                        nc.scalar.activation(
                            ow[:, mi * 512:(mi + 1) * 512], accs[mi][:],
                            AF.Identity, bias=prm(6, m))
                    nc.sync.dma_start(
                        out_d[wave * 512:(wave + 1) * 512, :]
                        .rearrange("(a p) t -> p a t", p=128),
                        ow[:].rearrange("p (a t) -> p a t", a=4))

    nc.compile()
    return nc


def _get_compiled():
    global _COMPILED
    if _COMPILED is None:
        _COMPILED = _build()
    return _COMPILED


def _host_prep(x, qkv_w, qkv_b, qn_w, qn_b, kn_w, kn_b, proj_w, proj_b):
    import ml_dtypes
    bf16 = ml_dtypes.bfloat16

    qkv_w = np.asarray(qkv_w, np.float32)
    proj_w = np.asarray(proj_w, np.float32)
    qkv_b = np.asarray(qkv_b, np.float32)
    qkvwT = np.ascontiguousarray(qkv_w.T).astype(bf16)
    projwT = np.ascontiguousarray(proj_w.T).astype(bf16)

    ws_q = qkv_w[0:C].sum(axis=0)
    ws_k = qkv_w[C:2 * C].sum(axis=0)
    wsums = np.zeros((128, 16), np.float32)
    for kt in range(8):
        wsums[:, kt * 2] = ws_q[kt * 128:(kt + 1) * 128]
        wsums[:, kt * 2 + 1] = ws_k[kt * 128:(kt + 1) * 128]

    # constf: params (7 groups x 8 cols) + bsum at col 56
    constf = np.zeros((128, 58), np.float32)
    proj_b2 = np.asarray(proj_b, np.float32) + proj_w @ qkv_b[2 * C:3 * C]
    for g, vec in enumerate([qn_w, qn_b, kn_w, kn_b,
                             qkv_b[0:C], qkv_b[C:2 * C], proj_b2]):
        constf[:, g * 8:(g + 1) * 8] = \
            np.asarray(vec, np.float32).reshape(8, 128).T
    constf[0, 56] = qkv_b[0:C].sum() / C
    constf[1, 56] = qkv_b[C:2 * C].sum() / C

    # constb: wsums(16) | sel(4)
    constb = np.zeros((128, 20), np.float32)
    constb[:, 0:16] = wsums
    constb[:, 16] = 1.0   # sel col0: all-ones -> partition sum into row 0
    constb[:, 19] = 1.0   # sel col3: all-ones -> partition sum into row 1
    constb = constb.astype(bf16)

    # selc rows: stats row selectors
    selc = np.zeros((2, 256), np.float32)
    selc[0, 0:128] = 1.0
    selc[1, 128:256] = 1.0

    in_maps = []
    for c in range(N_CORES):
        b, half = c // 2, c % 2
        xb = np.asarray(x[b], np.float32)
        xr = np.roll(xb, -half * TL, axis=0)   # local tokens -> rows [0,512)
        xT = np.ascontiguousarray(xr.T).astype(bf16)
        in_maps.append({
            "xT": xT, "qkvwT": qkvwT, "projwT": projwT,
            "constf": constf, "constb": constb, "selc": selc,
        })
    return in_maps


def _run(inputs, trace=False):
    from concourse.bass_utils import run_bass_kernel_spmd
    nc = _get_compiled()
    in_maps = _host_prep(**inputs)
    res = run_bass_kernel_spmd(nc, in_maps, core_ids=list(range(N_CORES)),
                               trace=trace)
    out = np.empty((B, N, C), np.float32)
    for c in range(N_CORES):
        b, half = c // 2, c % 2
        out[b, half * TL:(half + 1) * TL, :] = \
            np.asarray(res.results[c]["out"], np.float32).T
    return out, res


def kernel(**inputs):
    out, _ = _run(inputs, trace=False)
    return out


# revision 41
# speedup vs baseline: 1.0228x; 1.0228x over previous
"""Trainium2 Bass kernel for nn_Attention (dense transformer attention w/ QK-LayerNorm).

Sharding: sequence-parallel over 8 cores. Core c handles batch b = c//2,
token half h = c%2 (512 tokens). K/V are computed redundantly for the full
batch element on both cores of a pair; Q only for local tokens. No collectives.

Structure (v4):
 - bf16 on the SBUF side everywhere (weights, x, K/Q/V, probs, output);
   PSUM accumulation fp32. Halves DMA bytes, enables fast DVE modes + FWL.
 - Inputs arrive as 13 large DMAs (x, 2 halves per weight tensor); K/Q run
   kt-outer over 4-bank accumulator half-waves so matmuls start as soon as
   the first half-tensor lands (~7us) instead of after the full load.
 - All three LayerNorm stat chains share ONE Ln + ONE Exp ACT call -> no
   activation-table thrash against the attention-phase Exp stream.
 - Softmax exp: one ACT call per key-tile over a 2-bank [128,1024] PSUM pair
   (both heads of the pair at once).
 - Denominators: V carries a trailing ones-column per head, PV emits row 64 =
   sum(P); per-head-pair DVE reciprocal_approx_fast (at partition 0 only -
   the custom op miscompiles at partition base != 0) + GpSimd
   partition_broadcast of 1/d; osb scaled in-loop so the projection starts
   right after the last head pair.
 - PSUM: QKV scope = 4-bank acc rotation + {ssqK0,ssqK1,ssqQ,bcast};
   attention scope = 2x2-bank S pairs + O banks; proj scope = 4 banks.

Engine partition rule: writes and ACT reads start at 32-aligned partitions;
only DVE reads use unaligned bases.
"""

import numpy as np

B, N, C = 4, 1024, 1024
H, D = 16, 64
LN_EPS = 1e-5
N_CORES = 8
TL = 512          # local tokens per core
KT = 8            # channel tiles of 128
SCALE = D ** -0.5

_COMPILED = None


def _build():
    import concourse.bacc as bacc
    import concourse.tile as tile
    import concourse.mybir as mybir

    F32 = mybir.dt.float32
    F32R = mybir.dt.float32r
    BF16 = mybir.dt.bfloat16
    I16 = mybir.dt.int16
    AF = mybir.ActivationFunctionType
    OP = mybir.AluOpType
    # Schraudolph exp in bf16-bit space: int16(A*S + B) bitcast to bf16
    # approximates exp(SCALE*S); the constant-scale part of the error
    # cancels in the softmax normalization.
    EXPA = 128.0 * 1.4426950408889634 * SCALE
    EXPB = 128.0 * (127.0 - 0.0434)

    nc = bacc.Bacc("TRN2", target_bir_lowering=False, debug=False,
                   num_devices=N_CORES)

    xT_d = nc.dram_tensor("xT", [C, N], BF16, kind="ExternalInput").ap()
    qkvwT_d = nc.dram_tensor("qkvwT", [C, 3 * C], BF16, kind="ExternalInput").ap()
    projwT_d = nc.dram_tensor("projwT", [C, C], BF16, kind="ExternalInput").ap()
    constf_d = nc.dram_tensor("constf", [128, 58], F32, kind="ExternalInput").ap()
    constb_d = nc.dram_tensor("constb", [128, 20], BF16, kind="ExternalInput").ap()
    selc_d = nc.dram_tensor("selc", [2, 256], F32R, kind="ExternalInput").ap()
    out_d = nc.dram_tensor("out", [C, TL], BF16, kind="ExternalOutput").ap()

    with tile.TileContext(nc) as tc:
        with tc.tile_pool(name="persist", bufs=1) as pers, \
             tc.tile_pool(name="sq", bufs=2) as sqp, \
             tc.tile_pool(name="small", bufs=1) as smallp:

            khat = pers.tile([128, KT * N], BF16, tag="khat")       # 2MB
            vful = pers.tile([128, 8 * 1040], BF16, tag="vful")     # 2.03MB
            qhat = pers.tile([128, KT * TL], BF16, tag="qhat")      # 1MB
            osb = pers.tile([128, KT * TL], BF16, tag="osb")        # 1MB
            wpbig = pers.tile([128, KT * 1024], BF16, tag="wpbig")  # 2MB

            constf = smallp.tile([128, 58], F32, tag="constf")
            constb = smallp.tile([128, 20], BF16, tag="constb")
            selc = smallp.tile([2, 256], F32R, tag="selc")
            st = smallp.tile([2, 8 * 512], F32, tag="st")
            stmm = smallp.tile([2, 6 * 512], F32R, tag="stmm")
            rstdb = smallp.tile([128, 2 * N + 2 * TL], BF16, tag="rstdb")
            dd = smallp.tile([1, 1024], F32, tag="dd")
            ddr = smallp.tile([1, 1024], F32, tag="ddr")
            ddrb = smallp.tile([1, 1024], BF16, tag="ddrb")
            qzs = [smallp.tile([128, 1024], BF16, tag=f"qz{i}",
                               name=f"qz{i}") for i in range(2)]

            bsum = constf[0:2, 56:57]
            wsums = constb[:, 0:16]
            sel = constb[:, 16:20]    # col0 = sum-to-row-0 ; col3 = sum-to-row-1

            def sl(i):
                return st[:, i * 512:(i + 1) * 512]

            def smm(i):
                return stmm[:, i * 512:(i + 1) * 512]

            def prm(grp, kt):  # qn_w qn_b kn_w kn_b qb kb projb
                return constf[:, grp * 8 + kt: grp * 8 + kt + 1]

            nc.sync.dma_start(constf[:], constf_d[:])
            nc.sync.dma_start(constb[:], constb_d[:])
            nc.sync.dma_start(selc[:], selc_d[:])
            for q in qzs:
                nc.vector.memset(q[:], 0.0)

            with tc.tile_pool(name="xp", bufs=1) as xpool, \
                 tc.tile_pool(name="ntmp", bufs=3) as ntp, \
                 tc.tile_pool(name="qkvps", bufs=1, space="PSUM") as qps, \
                 tc.tile_pool(name="statps", bufs=1, space="PSUM") as sps:
                xT = xpool.tile([128, KT * N], BF16, tag="xT")      # 2MB
                wkbig = xpool.tile([128, KT * 1024], BF16, tag="wkbig")
                wqbig = xpool.tile([128, KT * 1024], BF16, tag="wqbig")
                wvbig = xpool.tile([128, KT * 1024], BF16, tag="wvbig")

                # ---- input DMA: xT and the K weights arrive in kt-halves,
                # interleaved, so the kt-outer K waves start on the first
                # 1.5MB instead of the full load; later tensors in g-halves
                xdst = xT[:].rearrange("p (a t) -> p a t", a=KT)
                xsrc = xT_d.rearrange("(a p) t -> p a t", p=128)
                wkdst = wkbig[:].rearrange("p (a t) -> p a t", a=KT)
                wksrc = qkvwT_d[:, C:2 * C].rearrange("(a p) t -> p a t",
                                                      p=128)

                def wload(big, col0):
                    for g in range(2):
                        dst = big[:].rearrange("p (a t) -> p a t", a=KT)
                        nc.sync.dma_start(
                            dst[:, :, g * 512:(g + 1) * 512],
                            qkvwT_d[:, col0 + g * 512: col0 + (g + 1) * 512]
                            .rearrange("(a p) t -> p a t", p=128))

                for h in range(2):
                    ks = slice(h * 4, (h + 1) * 4)
                    nc.sync.dma_start(xdst[:, ks, :], xsrc[:, ks, :])
                    nc.sync.dma_start(
                        wkdst[:, ks, 0:512], wksrc[:, ks, 0:512])
                for h in range(2):
                    ks = slice(h * 4, (h + 1) * 4)
                    nc.sync.dma_start(
                        wkdst[:, ks, 512:1024], wksrc[:, ks, 512:1024])
                wload(wqbig, 0)
                wload(wvbig, 2 * C)
                for g in range(2):
                    dst = wpbig[:].rearrange("p (a t) -> p a t", a=KT)
                    nc.sync.dma_start(
                        dst[:, :, g * 512:(g + 1) * 512],
                        projwT_d[:, g * 512:(g + 1) * 512]
                        .rearrange("(a p) t -> p a t", p=128))

                # ---- K phase: kt-outer half-waves of 4 accumulators ----
                def kq_wave(big, grp, moving0, mwidth, dst, dstep, bias_g):
                    accs = [qps.tile([128, mwidth], F32, tag="acc", bufs=4,
                                     name=f"acc{grp}_{mi}") for mi in range(4)]
                    for kt in range(KT):
                        for mi in range(4):
                            nc.tensor.matmul(
                                accs[mi][:],
                                big[:, kt * 1024 + grp * 512 + mi * 128:
                                    kt * 1024 + grp * 512 + (mi + 1) * 128],
                                xT[:, kt * N + moving0: kt * N + moving0 + mwidth],
                                start=(kt == 0), stop=(kt == KT - 1),
                                skip_group_check=True)
                    for mi in range(4):
                        m = grp * 4 + mi
                        nc.scalar.activation(
                            dst(m), accs[mi][:], AF.Identity,
                            bias=prm(bias_g, m))

                for grp in range(2):
                    for nh in range(2):
                        kq_wave(wkbig, grp, nh * 512, 512,
                                lambda m, nh=nh: khat[:, m * N + nh * 512:
                                                      m * N + (nh + 1) * 512],
                                512, 5)

                # ---- token sums of (q,k): needs only xT ----
                sums_ps = [qps.tile([2, 512], F32, tag="acc", bufs=4,
                                    name=f"sums_ps{i}") for i in range(2)]
                for kt in range(KT):
                    for nh in range(2):
                        nc.tensor.matmul(
                            sums_ps[nh][:], wsums[:, kt * 2: kt * 2 + 2],
                            xT[:, kt * N + nh * 512: kt * N + (nh + 1) * 512],
                            start=(kt == 0), stop=(kt == KT - 1),
                            skip_group_check=True)
                nc.vector.tensor_copy(sl(0), sums_ps[0][:])
                nc.vector.tensor_copy(sl(1), sums_ps[1][:])

                for grp in range(2):
                    kq_wave(wqbig, grp, 0, TL,
                            lambda m: qhat[:, m * TL:(m + 1) * TL], 512, 4)

                # ---- ssq matmuls (squares on DVE, overlap Q/V matmuls) ----
                ssq_ps = [sps.tile([2, 512], F32, tag=f"sq{i}", bufs=1,
                                   name=f"ssq_ps{i}") for i in range(2)]
                for m in range(8):
                    s = khat[:, m * N:(m + 1) * N]
                    ksq = sqp.tile([128, N], BF16, tag="sq")
                    nc.vector.tensor_mul(ksq[:], s, s)
                    for nh in range(2):
                        nc.tensor.matmul(
                            ssq_ps[nh][:], sel[:, 2:4],
                            ksq[:, nh * 512:(nh + 1) * 512],
                            start=(m == 0), stop=(m == 7),
                            skip_group_check=True)
                ssqQ_ps = sps.tile([2, 512], F32, tag="sqQ", bufs=1,
                                   name="ssqQ_ps")
                for m in range(8):
                    s = qhat[:, m * TL:(m + 1) * TL]
                    qsq = sqp.tile([128, TL], BF16, tag="sqq")
                    nc.vector.tensor_mul(qsq[:], s, s)
                    nc.tensor.matmul(ssqQ_ps[:], sel[:, 0:2], qsq[:],
                                     start=(m == 0), stop=(m == 7),
                                     skip_group_check=True)

                # ---- stats: one batched Ln+Exp for all three chains ----
                # slots: mu_a=sl(2) [nh0], mu_b=sl(3) [nh1]; vars at sl(4..6)
                # (nh0-k, nh1-k, q); rstd_cat -> smm(0..2), murs -> smm(3..5)
                nc.vector.tensor_scalar(sl(2), sl(0), 1.0 / C, bsum,
                                        OP.mult, OP.add)
                nc.vector.tensor_scalar(sl(3), sl(1), 1.0 / C, bsum,
                                        OP.mult, OP.add)

                def var_into(slot, ssq_src, mu):
                    nc.vector.tensor_copy(sl(7), ssq_src)
                    nc.vector.tensor_scalar(slot, sl(7), 1.0 / C, LN_EPS,
                                            OP.mult, OP.add)
                    nc.vector.tensor_mul(sl(7), mu, mu)
                    nc.vector.tensor_sub(slot, slot, sl(7))

                var_into(sl(4), ssq_ps[0][:], sl(2))
                var_into(sl(5), ssq_ps[1][:], sl(3))
                var_into(sl(6), ssqQ_ps[:], sl(2))
                vcat = st[:, 4 * 512:7 * 512]
                nc.vector.tensor_scalar_max(vcat, vcat, 1e-20)
                nc.scalar.activation(vcat, vcat, AF.Ln)
                nc.scalar.activation(stmm[:, 0:3 * 512], vcat, AF.Exp,
                                     scale=-0.5)
                nc.vector.tensor_mul(smm(3), sl(2), smm(0).bitcast(F32))
                nc.vector.tensor_mul(smm(4), sl(3), smm(1).bitcast(F32))
                nc.vector.tensor_mul(smm(5), sl(2), smm(2).bitcast(F32))

                def bcast(slot, row, dst_col):
                    bc_ps = sps.tile([128, 512], F32, tag="bc", bufs=1,
                                     name="bc_ps")
                    nc.tensor.matmul(bc_ps[:],
                                     selc[0:2, row * 128:(row + 1) * 128],
                                     smm(slot), start=True, stop=True)
                    nc.vector.tensor_copy(rstdb[:, dst_col:dst_col + 512],
                                          bc_ps[:])

                bcast(0, 1, 0)        # rstd_k nh0
                bcast(3, 1, N)        # murs_k nh0
                bcast(1, 1, 512)      # rstd_k nh1
                bcast(4, 1, N + 512)  # murs_k nh1
                bcast(2, 0, 2 * N)    # rstd_q
                bcast(5, 0, 2 * N + TL)

                # ---- V phase (kt-inner, acc rotation) ----
                for nh in range(2):
                    for mt in range(8):
                        base = mt * 1040
                        acc = qps.tile([128, 512], F32, tag="acc", bufs=4,
                                       name="vacc")
                        for kt in range(KT):
                            nc.tensor.matmul(
                                acc[:],
                                xT[:, kt * N + mt * 128: kt * N + (mt + 1) * 128],
                                wvbig[:, kt * 1024 + nh * 512:
                                      kt * 1024 + (nh + 1) * 512],
                                start=(kt == 0), stop=(kt == KT - 1))
                        # head slot layout: [64 V chans | ones]
                        dst = vful[:, base + nh * 8 * 65: base + (nh + 1) * 8 * 65]
                        nc.scalar.activation(
                            dst.rearrange("p (h e) -> p h e", h=8)[:, :, 0:64],
                            acc[:].rearrange("p (h e) -> p h e", h=8),
                            AF.Copy)
                for mt in range(8):
                    oc = vful[:, mt * 1040: (mt + 1) * 1040]
                    oc = oc.rearrange("p (h e) -> p h e", h=16)[:, :, 64:65]
                    nc.vector.memset(oc, 1.0)

                # ---- normalize K and Q (DVE+ACT, overlaps V matmuls) ----
                for m in range(8):
                    s = khat[:, m * N:(m + 1) * N]
                    t = ntp.tile([128, N], BF16, tag="nt")
                    nc.vector.tensor_mul(t[:], s, rstdb[:, 0:N])
                    nc.vector.tensor_sub(t[:], t[:], rstdb[:, N:2 * N])
                    nc.scalar.activation(s, t[:], AF.Identity,
                                         scale=prm(2, m), bias=prm(3, m))
                for m in range(8):
                    s = qhat[:, m * TL:(m + 1) * TL]
                    t = ntp.tile([128, TL], BF16, tag="ntq")
                    nc.vector.tensor_mul(t[:], s, rstdb[:, 2 * N:2 * N + TL])
                    nc.vector.tensor_sub(
                        t[:], t[:], rstdb[:, 2 * N + TL:2 * N + 2 * TL])
                    nc.scalar.activation(s, t[:], AF.Identity,
                                         scale=prm(0, m), bias=prm(1, m))

            # ---------- attention (head pairs, software-pipelined) ----------
            with tc.tile_pool(name="pp", bufs=3) as ppool, \
                 tc.tile_pool(name="att", bufs=2) as attp, \
                 tc.tile_pool(name="ot", bufs=2) as otp, \
                 tc.tile_pool(name="attps", bufs=1, space="PSUM") as aps:

                def qz_copy(kth):
                    qz = qzs[kth % 2]
                    nc.vector.tensor_copy(
                        qz[0:64, 0:512], qhat[0:64, kth * TL:(kth + 1) * TL])
                    nc.vector.tensor_copy(
                        qz[64:128, 512:1024],
                        qhat[64:128, kth * TL:(kth + 1) * TL])

                qz_copy(0)
                qz_copy(1)
                o_pss = {}
                rbs = {}

                def scale_osb(kth):
                    # fused drain+scale of head pair kth: reads its O banks
                    # (freeing them for kth+2) and its 1/d broadcast, which
                    # by now has had a full iteration to land
                    oA, oB = o_pss.pop(kth)
                    rb = rbs.pop(kth)
                    sl_o = osb[:, kth * TL:(kth + 1) * TL]
                    nc.vector.scalar_tensor_tensor(
                        sl_o[0:64, :], oA[0:64, :], 1.0, rb[0:64, 0:512],
                        OP.mult, OP.mult)
                    nc.vector.scalar_tensor_tensor(
                        sl_o[64:128, :], oB[0:64, :], 1.0,
                        rb[64:128, 512:1024], OP.mult, OP.mult)

                for kth in range(8):
                    hA, hB = 2 * kth, 2 * kth + 1
                    qz = qzs[kth % 2]
                    o_psA = aps.tile([65, 512], F32, tag="oA", bufs=2,
                                     name=f"oA{kth}")
                    o_psB = aps.tile([65, 512], F32, tag="oB", bufs=2,
                                     name=f"oB{kth}")

                    s_tiles = {}

                    def emit_S(tt, qz=qz, kth=kth, s_tiles=s_tiles):
                        ksl = khat[:, kth * N + tt * 128: kth * N + (tt + 1) * 128]
                        sp = aps.tile([128, 1024], F32, tag="sp", bufs=2,
                                      name=f"sp{tt}")
                        nc.tensor.matmul(sp[:, 0:512], ksl, qz[:, 0:512],
                                         start=True, stop=True)
                        nc.tensor.matmul(sp[:, 512:1024], ksl, qz[:, 512:1024],
                                         start=True, stop=True)
                        s_tiles[tt] = sp

                    emit_S(0)
                    emit_S(1)
                    for tt in range(8):
                        sp = s_tiles.pop(tt)
                        pe = ppool.tile([128, 1024], BF16, tag="p", name="pe")
                        nc.scalar.activation(pe[:], sp[:], AF.Exp, scale=SCALE)
                        if tt + 2 < 8:
                            emit_S(tt + 2)
                        nc.tensor.matmul(
                            o_psA[:],
                            vful[:, tt * 1040 + hA * 65: tt * 1040 + (hA + 1) * 65],
                            pe[:, 0:512], start=(tt == 0), stop=(tt == 7))
                        nc.tensor.matmul(
                            o_psB[:],
                            vful[:, tt * 1040 + hB * 65: tt * 1040 + (hB + 1) * 65],
                            pe[:, 512:1024], start=(tt == 0), stop=(tt == 7))

                    # prefetch next-next qz so S(kth+1) never waits on DVE
                    if kth + 2 < 8:
                        qz_copy(kth + 2)
                    # denominators -> 1/d (DVE) -> broadcast (GpSimd)
                    nc.vector.tensor_copy(dd[0:1, 0:512], o_psA[64:65, :])
                    nc.vector.tensor_copy(dd[0:1, 512:1024], o_psB[64:65, :])
                    nc.vector.reciprocal_approx_fast(ddr[0:1, :], dd[0:1, :])
                    nc.vector.tensor_copy(ddrb[:], ddr[:])
                    rb = attp.tile([128, 1024], BF16, tag="rb")
                    nc.gpsimd.partition_broadcast(rb[:], ddrb[0:1, :])
                    o_pss[kth] = (o_psA, o_psB)
                    rbs[kth] = rb
                    if kth > 0:
                        scale_osb(kth - 1)
                scale_osb(7)

                # ---------- output projection (same PSUM scope: the acc
                # tiles recycle the sp/oA/oB bank slots, so no scope barrier
                # separates the last head pair from the projection) ----------
                for wave in range(2):
                    accs = [aps.tile([128, 512], F32, tag=t, bufs=b,
                                     name=f"pacc{wave}_{mi}")
                            for mi, (t, b) in enumerate(
                                (("sp", 2), ("sp", 2), ("oA", 2), ("oB", 2)))]
                    for kth in range(8):
                        for mi in range(4):
                            m = wave * 4 + mi
                            nc.tensor.matmul(
                                accs[mi][:],
                                wpbig[:, kth * 1024 + (m // 4) * 512 + (m % 4) * 128:
                                      kth * 1024 + (m // 4) * 512 + (m % 4 + 1) * 128],
                                osb[:, kth * TL:(kth + 1) * TL],
                                start=(kth == 0), stop=(kth == 7),
                                skip_group_check=True)
                    ow = otp.tile([128, 4 * 512], BF16, tag="ot",
                                  name=f"ow{wave}")
                    for mi in range(4):
                        m = wave * 4 + mi
                        nc.scalar.activation(
                            ow[:, mi * 512:(mi + 1) * 512], accs[mi][:],
                            AF.Identity, bias=prm(6, m))
                    nc.sync.dma_start(
                        out_d[wave * 512:(wave + 1) * 512, :]
                        .rearrange("(a p) t -> p a t", p=128),
                        ow[:].rearrange("p (a t) -> p a t", a=4))

    nc.compile()
    return nc


def _get_compiled():
    global _COMPILED
    if _COMPILED is None:
        _COMPILED = _build()
    return _COMPILED


def _host_prep(x, qkv_w, qkv_b, qn_w, qn_b, kn_w, kn_b, proj_w, proj_b):
    import ml_dtypes
    bf16 = ml_dtypes.bfloat16

    qkv_w = np.asarray(qkv_w, np.float32)
    proj_w = np.asarray(proj_w, np.float32)
    qkv_b = np.asarray(qkv_b, np.float32)
    qkvwT = np.ascontiguousarray(qkv_w.T).astype(bf16)
    projwT = np.ascontiguousarray(proj_w.T).astype(bf16)

    ws_q = qkv_w[0:C].sum(axis=0)
    ws_k = qkv_w[C:2 * C].sum(axis=0)
    wsums = np.zeros((128, 16), np.float32)
    for kt in range(8):
        wsums[:, kt * 2] = ws_q[kt * 128:(kt + 1) * 128]
        wsums[:, kt * 2 + 1] = ws_k[kt * 128:(kt + 1) * 128]

    # constf: params (7 groups x 8 cols) + bsum at col 56
    constf = np.zeros((128, 58), np.float32)
    proj_b2 = np.asarray(proj_b, np.float32) + proj_w @ qkv_b[2 * C:3 * C]
    for g, vec in enumerate([qn_w, qn_b, kn_w, kn_b,
                             qkv_b[0:C], qkv_b[C:2 * C], proj_b2]):
        constf[:, g * 8:(g + 1) * 8] = \
            np.asarray(vec, np.float32).reshape(8, 128).T
    constf[0, 56] = qkv_b[0:C].sum() / C
    constf[1, 56] = qkv_b[C:2 * C].sum() / C

    # constb: wsums(16) | sel(4)
    constb = np.zeros((128, 20), np.float32)
    constb[:, 0:16] = wsums
    constb[:, 16] = 1.0   # sel col0: all-ones -> partition sum into row 0
    constb[:, 19] = 1.0   # sel col3: all-ones -> partition sum into row 1
    constb = constb.astype(bf16)

    # selc rows: stats row selectors
    selc = np.zeros((2, 256), np.float32)
    selc[0, 0:128] = 1.0
    selc[1, 128:256] = 1.0

    in_maps = []
    for c in range(N_CORES):
        b, half = c // 2, c % 2
        xb = np.asarray(x[b], np.float32)
        xr = np.roll(xb, -half * TL, axis=0)   # local tokens -> rows [0,512)
        xT = np.ascontiguousarray(xr.T).astype(bf16)
        in_maps.append({
            "xT": xT, "qkvwT": qkvwT, "projwT": projwT,
            "constf": constf, "constb": constb, "selc": selc,
        })
    return in_maps


def _run(inputs, trace=False):
    from concourse.bass_utils import run_bass_kernel_spmd
    nc = _get_compiled()
    in_maps = _host_prep(**inputs)
    res = run_bass_kernel_spmd(nc, in_maps, core_ids=list(range(N_CORES)),
                               trace=trace)
    out = np.empty((B, N, C), np.float32)
    for c in range(N_CORES):
        b, half = c // 2, c % 2
        out[b, half * TL:(half + 1) * TL, :] = \
            np.asarray(res.results[c]["out"], np.float32).T
    return out, res


def kernel(**inputs):
    out, _ = _run(inputs, trace=False)
    return out
